# revision 40
# baseline (speedup 1.0000x reference)
"""TRN2 Bass kernel for nn_DeepFeatureLoss (B=4, N=4096, D=64, 8 cores).

Math (per batch b):
  P = softmax_j(-|x_i/s - x_j/s|^2)        (spatial gaussian, s=0.05)
  Q = softmax_j(-|f1_i - f2_j|^2)          (feature affinity)
  loss_b = sum_i w_i * sum_j (P_ij - Q_ij)^2

Key structural ideas (v2, banded points):
  - sigma=0.05 makes P extremely peaked: only spatially-near pairs matter.
    The host sorts each batch's points with reverse Cuthill-McKee on the
    significant-pair graph (score >= rowmax - 15), which empirically gives
    bandwidth ~220 on this data. After sorting, P is (nearly) banded.
  - Each core gets 512 rows; its column space is ROTATED (host-side roll)
    so its rows sit at rotated columns [256, 768). Band windows then become
    core-independent diagonal windows of width W=448 centred on each row
    tile, so one SPMD program serves all 8 cores. Column rotation is
    harmless: softmax row-sums are permutation-invariant. A quantitative
    dropped-mass bound selects W=1024 fallback if the data needs it.
  - e_p is computed only on the band (ACT exp of a [128, W] psum tile) and
    zero-extended into a full-width buffer (Pool memsets the outside), so
    the fused DVE op is identical to the full version:
       li = sum_j (e_p - rho*e_f)^2 * (w/S_p^2),  rho = S_p/S_f.
  - Features use ONE K=128 bf16 matmul pass (64 hi-product rows + 60
    lo-correction rows + 2 g2 rows + 2 bias rows); validated 1.7e-4 final
    rel err vs fp64 (tolerance 2e-2).
  - Per row tile ACT does 4 exp instructions (1536+1536+1024 feature chunks
    + W band) = 4096+W elems vs 8192 in the full version; PE does 4096+W
    cols vs 12288. S_p comes from a DVE reduce (ACT accum only on the final
    tile, where DVE saturation would put the reduce on the tail path).
"""

import os
import numpy as np
import ml_dtypes

bf16 = ml_dtypes.bfloat16

SIGMA = 0.05
SHIFT_F = 30.0  # lifts feature exp away from denormal S_f; validated on data
RCM_DELTA = 8.0  # significant-pair cutoff for the ordering graph; a tight
# graph makes RCM pack the truly-massive pairs into a narrower band

B, N, D = 4, 4096, 64
NCORES = 8
SHARD = N // NCORES          # 512 rows per core per batch
RT_PER_BATCH = SHARD // 128  # 4 row tiles of 128
ROT_BASE = 256               # core rows sit at rotated cols [256, 768)

KP = 24    # points: 18 coord-split rows + 3 y2 rows + 3 bias rows
KF = 128   # features: 64 hi + NLO lo-corrections + 2 g2 + 2 bias
NLO = 60

_cache = {}

_last_results = None  # stashed BassKernelResults for test harnesses


def _get_sqdiff_op():
    """Register (once) a fused DVE op:
        out = (in0 - in1*s0)^2 * s1 ; accum_out = sum_k out[k]
    i.e. li = sum_j (e_p - rho*e_f)^2 * (w/S_p^2) in ONE 1x DVE pass.
    """
    if "sqdiff" in _cache:
        return _cache["sqdiff"]
    import re
    from operator import add as _add
    from concourse import dve_ops
    from concourse.dve_spec import Spec, Src0, Src1, C0, C1, Zero, sq

    name = "SQDIFF_SCALE_RED_DFL"

    def _ref(in0, in1, s0, s1, imm2):
        b = (((in0.astype(np.float32) - in1 * s0) ** 2) * s1).astype(np.float32)
        return b, b.reshape(b.shape[0], -1).sum(axis=-1, keepdims=True)

    spec = Spec(body=sq(Src0 - Src1 * C0) * C1, accum=_add, accum_init=Zero,
                reference=_ref)
    if name not in dve_ops._SUB_OPCODE_FOR_NAME:
        row = max(dve_ops._SUB_OPCODE_FOR_NAME.values()) + 1
        assert row < 0x20
        dve_ops._SUB_OPCODE_FOR_NAME[name] = row
    shas = {}
    for ver in ("v3", "v4"):
        probe = dve_ops.DveOp(name, spec, subdim=False, uops_sha={})
        try:
            probe.compile(ver)
        except ValueError as e:
            m = re.search(r"\{ver\}.*?=\"([0-9a-f]+)\"".replace("{ver}", ver),
                          str(e)) or re.search(r'"([0-9a-f]{16})"', str(e))
            shas[ver] = m.group(1)
    op = dve_ops.DveOp(name, spec, subdim=False, uops_sha=shas)
    if not any(o.name == name for o in dve_ops.OPS):
        dve_ops.OPS.append(op)
    dve_ops.CUSTOM_DVE_SPECS[name] = spec
    _cache["sqdiff"] = op
    return op


def _band_c0(rt, w, n=N):
    """Rotated-space band window start for local row tile rt."""
    center = ROT_BASE + rt * 128 + 64
    return max(0, min(n - w, center - w // 2))


def _build_program(w, n=N, shard=SHARD, nb=B):
    """Emit the per-core Bass program. Identical on all cores (pure SPMD)."""
    import concourse.bacc as bacc
    import concourse.tile as tile
    from concourse import mybir

    f32 = mybir.dt.float32
    b16 = mybir.dt.bfloat16
    AX = mybir.AxisListType
    ACTF = mybir.ActivationFunctionType

    rts = shard // 128
    chunks = [(0, 1536), (1536, 3072), (3072, 4096)]
    bd_bufs = 2 if w <= 512 else 1  # psum: 6 banks for chunks + band

    nc = bacc.Bacc("TRN2", target_bir_lowering=False, debug=False,
                   num_devices=NCORES)

    RP = nc.dram_tensor("rp", [nb, KP, n], b16, kind="ExternalInput").ap()
    RF = nc.dram_tensor("rf", [nb, KF, n], b16, kind="ExternalInput").ap()
    # combined lhsT: cols [0:shard] = lf (128 rows), cols [shard:2*shard] =
    # lp (rows 0:KP, rest zero-padded) -> one DMA per batch
    LPF = nc.dram_tensor("lpf", [nb, 128, 2 * shard], b16,
                         kind="ExternalInput").ap()
    WV = nc.dram_tensor("wv", [128, nb * rts], f32, kind="ExternalInput").ap()
    OUT = nc.dram_tensor("out", [128, nb * rts], f32,
                         kind="ExternalOutput").ap()

    with tile.TileContext(nc) as tc:
        with (
            tc.tile_pool(name="rhs", bufs=2) as rhs_pool,
            tc.tile_pool(name="lhs", bufs=2) as lhs_pool,
            tc.tile_pool(name="efb", bufs=3) as ef_pool,
            tc.tile_pool(name="epb", bufs=3) as ep_pool,
            tc.tile_pool(name="ubuf", bufs=2) as u_pool,
            tc.tile_pool(name="small", bufs=8) as small,
            tc.tile_pool(name="acc", bufs=1) as acc_pool,
            tc.tile_pool(name="psA", bufs=2, space="PSUM") as psA,
            tc.tile_pool(name="psB", bufs=bd_bufs, space="PSUM") as psB,
        ):
            li_cols = acc_pool.tile([128, nb * rts], f32)
            w_all = acc_pool.tile([128, nb * rts], f32)

            # dummy exp to hoist the ACT table load off the critical path
            warm = small.tile([1, 1], f32, tag="warm")
            nc.vector.memset(warm, 0.0)
            nc.scalar.activation(out=warm, in_=warm, func=ACTF.Exp)

            # PE p-state warm-up: get the tensor engine off the cold clock
            # while the first DMAs are in flight
            pe_w = acc_pool.tile([1, 512], b16)
            nc.vector.memset(pe_w, 0.0)
            wcols = min(w, 512)
            for k in range(6):
                pwp = psB.tile([128, w], f32, tag="bd", name=f"pewarm{k}")
                nc.tensor.matmul(pwp[:, 0:wcols], pe_w[:, 0:128],
                                 pe_w[:, 0:wcols], start=True, stop=True)

            for b in range(nb):
                # lhsT + small band rhs first: they gate the first ACT inst
                lpf_t = lhs_pool.tile([128, 2 * shard], b16, tag="lpf")
                nc.sync.dma_start(out=lpf_t, in_=LPF[b])
                lf_t = lpf_t[:, 0:shard]
                lp_t = lpf_t[0:KP, shard:2 * shard]
                rp_t = rhs_pool.tile([KP, n], b16, tag="rp")
                nc.sync.dma_start(out=rp_t, in_=RP[b])
                # rf in 3 pieces matching the ACT chunks so chunk1 compute
                # starts as soon as its columns land
                rf_p = [rhs_pool.tile([KF, ce - cs], b16, tag=f"rf{pi}",
                                      name=f"rf_p{pi}")
                        for pi, (cs, ce) in enumerate(chunks)]
                for pi, (cs, ce) in enumerate(chunks):
                    nc.sync.dma_start(out=rf_p[pi], in_=RF[b][:, cs:ce])
                if b == 0:
                    # non-urgent; keep it off the critical first-RT DMA path
                    nc.sync.dma_start(out=w_all, in_=WV)

                for rt in range(rts):
                    r0 = rt * 128
                    c0 = _band_c0(rt, w, n)
                    col = b * rts + rt

                    e_f = ef_pool.tile([128, n], f32, tag="ef")
                    e_p = ep_pool.tile([128, n], f32, tag="ep")
                    sfc = small.tile([128, 4], f32, tag="sfc")
                    s_p = small.tile([128, 1], f32, tag="sp")

                    # zero the outside-band region of e_p on Pool
                    if c0 > 0:
                        nc.gpsimd.memset(e_p[:, 0:c0], 0.0)
                    if c0 + w < n:
                        nc.gpsimd.memset(e_p[:, c0 + w:n], 0.0)

                    # band: points matmul + exp (accum -> S_p)
                    pB = psB.tile([128, w], f32, tag="bd")
                    for q in range(0, w, 512):
                        qw = min(512, w - q)
                        nc.tensor.matmul(
                            pB[:, q:q + qw], lp_t[:, r0:r0 + 128],
                            rp_t[:, c0 + q:c0 + q + qw],
                            start=True, stop=True)
                    last_rt = (b == nb - 1 and rt == rts - 1)
                    if last_rt:
                        # keep S_p on the ACT accum for the final tile: the
                        # DVE is saturated at the tail and a reduce there
                        # would gate the last fused op
                        nc.scalar.activation(
                            out=e_p[:, c0:c0 + w], in_=pB, func=ACTF.Exp,
                            scale=1.0, accum_out=s_p)
                    else:
                        nc.scalar.activation(
                            out=e_p[:, c0:c0 + w], in_=pB, func=ACTF.Exp,
                            scale=1.0)
                        # S_p via DVE (saves the ACT accum-read; DVE has slack)
                        nc.vector.reduce_sum(s_p, e_p[:, c0:c0 + w],
                                             axis=AX.X)

                    # feature chunks: matmul + exp (accums -> sfc cols)
                    for ci, (cs, ce) in enumerate(chunks):
                        width = ce - cs
                        pA = psA.tile([128, 1536], f32, tag="A",
                                      name=f"A{ci}")
                        for q in range(0, width, 512):
                            nc.tensor.matmul(
                                pA[:, q:q + 512], lf_t[:, r0:r0 + 128],
                                rf_p[ci][:, q:q + 512],
                                start=True, stop=True)
                        nc.scalar.activation(
                            out=e_f[:, cs:ce], in_=pA[:, 0:width],
                            func=ACTF.Exp, scale=1.0,
                            accum_out=sfc[:, ci:ci + 1])

                    s_f = small.tile([128, 1], f32, tag="sf")
                    nc.vector.reduce_sum(s_f, sfc[:, 0:3], axis=AX.X)
                    rsf = small.tile([128, 1], f32, tag="rsf")
                    nc.vector.reciprocal(rsf, s_f)
                    rho = small.tile([128, 1], f32, tag="rho")
                    nc.vector.tensor_mul(rho, s_p, rsf)
                    rsp = small.tile([128, 1], f32, tag="rsp")
                    nc.vector.reciprocal(rsp, s_p)
                    rsp2 = small.tile([128, 1], f32, tag="rsp2")
                    nc.vector.tensor_mul(rsp2, rsp, rsp)
                    s1v = small.tile([128, 1], f32, tag="s1v")
                    nc.vector.tensor_mul(s1v, rsp2, w_all[:, col:col + 1])

                    u_t = u_pool.tile([128, n], b16, tag="u")
                    nc.vector._custom_dve(
                        _get_sqdiff_op(), out=u_t, in0=e_p, in1=e_f,
                        s0=rho, s1=s1v, accum_out=li_cols[:, col:col + 1])

            # ship the per-row-tile partials; host does the final sums
            nc.sync.dma_start(out=OUT, in_=li_cols)

    nc.compile()
    return nc


def _split(x, levels):
    """Split fp array into `levels` bf16 terms (hi, mid, lo...)."""
    parts = []
    r = np.asarray(x, np.float64)
    for _ in range(levels):
        h = r.astype(np.float32).astype(bf16)
        parts.append(h)
        r = r - h.astype(np.float64)
    return parts


def _rcm_order(pts):
    """Reverse Cuthill-McKee order of the significant-pair graph."""
    from scipy.sparse import csr_matrix
    from scipy.sparse.csgraph import reverse_cuthill_mckee
    xs = (pts / SIGMA).astype(np.float32)
    x2 = (xs * xs).sum(-1)
    s = 2.0 * (xs @ xs.T) - x2[None, :]  # score + |x_i|^2 (row shift ok)
    smax = s.max(1, keepdims=True)
    sig = s >= (smax - np.float32(RCM_DELTA))
    adj = csr_matrix(sig | sig.T)
    return np.asarray(reverse_cuthill_mckee(adj, symmetric_mode=True))


def _band_geometry(pts_sorted):
    """Per-row exp-mass column cumsums for window coverage analysis."""
    xs = (pts_sorted / SIGMA).astype(np.float32)
    x2 = (xs * xs).sum(-1)
    s = 2.0 * (xs @ xs.T) - x2[None, :] - x2[:, None]
    smax = s.max(1, keepdims=True)
    e = np.exp((s - smax).astype(np.float64))
    return np.cumsum(e, axis=1), e.sum(1)


def _tile_bound(csum, tot, w_sorted, t, lo, hi):
    """4*sum w_i*eps_i for tile t with (unclamped) global window [lo, hi)."""
    lo_c, hi_c = max(0, lo), min(N, hi)
    rows = slice(t * 128, (t + 1) * 128)
    if hi_c <= lo_c:
        cov = np.zeros(128)
    else:
        cov = csum[rows, hi_c - 1] - (csum[rows, lo_c - 1] if lo_c > 0 else 0.0)
    eps = np.maximum(1.0 - cov / tot[rows], 0.0)
    return float((w_sorted[rows] * 4.0 * eps).sum())


def _core_shift(csum, tot, w_sorted, w, c, shard=SHARD):
    """Best per-core window shift (windows move together) + its bound."""
    best_s, best_b = 0, np.inf
    for s in range(-128, 129, 8):
        bc = 0.0
        for rt in range(shard // 128):
            t = (c * shard) // 128 + rt
            lo = t * 128 + 64 - w // 2 - s
            bc += _tile_bound(csum, tot, w_sorted, t, lo, lo + w)
        if bc < best_b:
            best_s, best_b = s, bc
    return best_s, best_b


def _band_err_bound(pts_sorted, w_sorted, w):
    """Zero-shift bound (fallback path check)."""
    csum, tot = _band_geometry(pts_sorted)
    return sum(_tile_bound(csum, tot, w_sorted, t,
                           t * 128 + 64 - w // 2, t * 128 + 64 + w // 2)
               for t in range(N // 128))


def _prep_inputs(points, weights, pointfea1, pointfea2,
                 nb=B, n=N, shard=SHARD, ncores=NCORES):
    """Host-side sort/shard/layout prep -> (w, list of per-core input dicts)."""
    points = np.asarray(points, np.float64)
    weights = np.asarray(weights, np.float64)
    f1 = np.asarray(pointfea1, np.float64)
    f2 = np.asarray(pointfea2, np.float64)

    rts = shard // 128
    per_batch = []
    geos = []
    for b in range(nb):
        order = _rcm_order(points[b])
        p = points[b][order]
        per_batch.append((order, p, weights[b][order], f1[b][order],
                          f2[b][order]))
        geos.append(_band_geometry(p))

    # smallest window whose per-core-optimally-shifted coverage bound passes
    w_band, shifts = 1024, np.zeros((nb, ncores), np.int64)
    for w_try in (256, 288, 320, 448):
        cand = np.zeros((nb, ncores), np.int64)
        bound = 0.0
        for b in range(nb):
            csum, tot = geos[b]
            for c in range(ncores):
                s, bc = _core_shift(csum, tot, per_batch[b][2], w_try, c)
                cand[b, c] = s
                bound += bc
        if bound < 1e-3:
            w_band, shifts = w_try, cand
            break
    if w_band > 512:
        for (order, p, w_, _, _) in per_batch:
            assert _band_err_bound(p, w_, w_band) < 5e-3, \
                "band window too narrow for data"

    rp_all = np.empty((nb, KP, n), bf16)
    rf_all = np.empty((nb, KF, n), bf16)
    lp_all = np.empty((nb, KP, n), bf16)   # full rows; sliced per core
    lf_all = np.empty((nb, KF, n), bf16)
    wv_all = np.empty((nb, n), np.float32)

    for b, (order, p, w_, a, c) in enumerate(per_batch):
        xs = p / SIGMA
        x2 = (xs * xs).sum(-1)
        xh, xm, xl = _split(xs, 3)
        y2h, y2m, y2l = _split(x2, 3)
        bph, bpm, bpl = _split(-x2, 3)

        for d in range(3):
            rrows = [2 * xh[..., d], 2 * xm[..., d], 2 * xh[..., d],
                     2 * xl[..., d], 2 * xh[..., d], 2 * xm[..., d]]
            lrows = [xh[..., d], xh[..., d], xm[..., d],
                     xh[..., d], xl[..., d], xm[..., d]]
            for k in range(6):
                rp_all[b, 6 * d + k] = rrows[k]
                lp_all[b, 6 * d + k] = lrows[k]
        rp_all[b, 18] = -y2h
        rp_all[b, 19] = -y2m
        rp_all[b, 20] = -y2l
        rp_all[b, 21:24] = np.float32(1.0)
        lp_all[b, 18:21] = np.float32(1.0)
        lp_all[b, 21] = bph
        lp_all[b, 22] = bpm
        lp_all[b, 23] = bpl

        g1 = (a * a).sum(-1)
        g2 = (c * c).sum(-1)
        f1h, f1l = _split(a, 2)
        f2h, _ = _split(c, 2)
        g2h, g2m = _split(g2, 2)
        bfh, bfm = _split(np.float64(SHIFT_F) - g1, 2)

        rf_all[b, :64] = (2 * f2h.astype(np.float32)).T
        rf_all[b, 64:64 + NLO] = (2 * f2h[:, :NLO].astype(np.float32)).T
        rf_all[b, 124] = -g2h
        rf_all[b, 125] = -g2m
        rf_all[b, 126:128] = np.float32(1.0)
        lf_all[b, :64] = f1h.T
        lf_all[b, 64:64 + NLO] = f1l[:, :NLO].T
        lf_all[b, 124:126] = np.float32(1.0)
        lf_all[b, 126] = bfh
        lf_all[b, 127] = bfm

        wv_all[b] = w_.astype(np.float32)

    in_maps = []
    for cidx in range(ncores):
        r0 = cidx * shard
        sl = slice(r0, r0 + shard)
        rp = np.empty_like(rp_all)
        rf = np.empty_like(rf_all)
        for b in range(nb):
            # per-(batch,core) roll: aligns this core's rows to the fixed
            # rotated-space windows, plus the data-optimized shift
            roll = ROT_BASE - r0 + int(shifts[b, cidx])
            rp[b] = np.roll(rp_all[b], roll, axis=1)
            rf[b] = np.roll(rf_all[b], roll, axis=1)
        rp = np.ascontiguousarray(rp)
        rf = np.ascontiguousarray(rf)
        lpf = np.zeros((nb, 128, 2 * shard), bf16)
        lpf[:, :, :shard] = lf_all[:, :, sl]
        lpf[:, :KP, shard:] = lp_all[:, :, sl]
        wvt = np.empty((128, nb * rts), np.float32)
        for b in range(nb):
            for rt in range(rts):
                wvt[:, b * rts + rt] = \
                    wv_all[b, r0 + rt * 128:r0 + (rt + 1) * 128]
        in_maps.append({"rp": rp, "rf": rf,
                        "lpf": np.ascontiguousarray(lpf), "wv": wvt})
    return w_band, in_maps


def kernel(points, weights, pointfea1, pointfea2):
    global _last_results
    from concourse.bass_utils import run_bass_kernel_spmd

    w_band, in_maps = _prep_inputs(points, weights, pointfea1, pointfea2)
    key = f"nc_w{w_band}"
    if key not in _cache:
        _cache[key] = _build_program(w_band)
        _cache["nc"] = _cache[key]  # for test harness TimelineSim
    nc = _cache[key]
    _cache["nc"] = nc

    res = run_bass_kernel_spmd(nc, in_maps, core_ids=list(range(NCORES)))
    _last_results = res
    rts = RT_PER_BATCH
    out = np.zeros(B, np.float64)
    for c in range(NCORES):
        li = np.asarray(res.results[c]["out"], np.float64)  # [128, B*rts]
        for b in range(B):
            out[b] += li[:, b * rts:(b + 1) * rts].sum()
    return out.astype(np.float32)


if __name__ == "__main__":
    rng = np.random.default_rng(0)
    pts = rng.random((B, N, 3), np.float32)
    w = rng.random((B, N), np.float32)
    w /= w.sum(1, keepdims=True)
    a = rng.standard_normal((B, N, D)).astype(np.float32)
    bfea = rng.standard_normal((B, N, D)).astype(np.float32)
    out = kernel(pts, w, a, bfea)
    print("kernel out:", out)


# revision 41
# speedup vs baseline: 1.0169x; 1.0169x over previous
"""TRN2 Bass kernel for nn_DeepFeatureLoss (B=4, N=4096, D=64, 8 cores).

Math (per batch b):
  P = softmax_j(-|x_i/s - x_j/s|^2)        (spatial gaussian, s=0.05)
  Q = softmax_j(-|f1_i - f2_j|^2)          (feature affinity)
  loss_b = sum_i w_i * sum_j (P_ij - Q_ij)^2

Key structural ideas (v2, banded points):
  - sigma=0.05 makes P extremely peaked: only spatially-near pairs matter.
    The host sorts each batch's points with reverse Cuthill-McKee on the
    significant-pair graph (score >= rowmax - 15), which empirically gives
    bandwidth ~220 on this data. After sorting, P is (nearly) banded.
  - Each core gets 512 rows; its column space is ROTATED (host-side roll)
    so its rows sit at rotated columns [256, 768). Band windows then become
    core-independent diagonal windows of width W=448 centred on each row
    tile, so one SPMD program serves all 8 cores. Column rotation is
    harmless: softmax row-sums are permutation-invariant. A quantitative
    dropped-mass bound selects W=1024 fallback if the data needs it.
  - e_p is computed only on the band (ACT exp of a [128, W] psum tile) and
    zero-extended into a full-width buffer (Pool memsets the outside), so
    the fused DVE op is identical to the full version:
       li = sum_j (e_p - rho*e_f)^2 * (w/S_p^2),  rho = S_p/S_f.
  - Features use ONE K=128 bf16 matmul pass (64 hi-product rows + 60
    lo-correction rows + 2 g2 rows + 2 bias rows); validated 1.7e-4 final
    rel err vs fp64 (tolerance 2e-2).
  - Per row tile ACT does 4 exp instructions (1536+1536+1024 feature chunks
    + W band) = 4096+W elems vs 8192 in the full version; PE does 4096+W
    cols vs 12288. S_p comes from a DVE reduce (ACT accum only on the final
    tile, where DVE saturation would put the reduce on the tail path).
"""

import os
import numpy as np
import ml_dtypes

bf16 = ml_dtypes.bfloat16

SIGMA = 0.05
SHIFT_F = 30.0  # lifts feature exp away from denormal S_f; validated on data
RCM_DELTA = 8.0  # significant-pair cutoff for the ordering graph; a tight
# graph makes RCM pack the truly-massive pairs into a narrower band

B, N, D = 4, 4096, 64
NCORES = 8
SHARD = N // NCORES          # 512 rows per core per batch
RT_PER_BATCH = SHARD // 128  # 4 row tiles of 128
ROT_BASE = 256               # core rows sit at rotated cols [256, 768)

KP = 24    # points: 18 coord-split rows + 3 y2 rows + 3 bias rows
KF = 128   # features: 64 hi + NLO lo-corrections + 2 g2 + 2 bias
NLO = 60

_cache = {}

_last_results = None  # stashed BassKernelResults for test harnesses


def _get_sqdiff_op():
    """Register (once) a fused DVE op:
        out = (in0 - in1*s0)^2 * s1 ; accum_out = sum_k out[k]
    i.e. li = sum_j (e_p - rho*e_f)^2 * (w/S_p^2) in ONE 1x DVE pass.
    """
    if "sqdiff" in _cache:
        return _cache["sqdiff"]
    import re
    from operator import add as _add
    from concourse import dve_ops
    from concourse.dve_spec import Spec, Src0, Src1, C0, C1, Zero, sq

    name = "SQDIFF_SCALE_RED_DFL"

    def _ref(in0, in1, s0, s1, imm2):
        b = (((in0.astype(np.float32) - in1 * s0) ** 2) * s1).astype(np.float32)
        return b, b.reshape(b.shape[0], -1).sum(axis=-1, keepdims=True)

    spec = Spec(body=sq(Src0 - Src1 * C0) * C1, accum=_add, accum_init=Zero,
                reference=_ref)
    if name not in dve_ops._SUB_OPCODE_FOR_NAME:
        row = max(dve_ops._SUB_OPCODE_FOR_NAME.values()) + 1
        assert row < 0x20
        dve_ops._SUB_OPCODE_FOR_NAME[name] = row
    shas = {}
    for ver in ("v3", "v4"):
        probe = dve_ops.DveOp(name, spec, subdim=False, uops_sha={})
        try:
            probe.compile(ver)
        except ValueError as e:
            m = re.search(r"\{ver\}.*?=\"([0-9a-f]+)\"".replace("{ver}", ver),
                          str(e)) or re.search(r'"([0-9a-f]{16})"', str(e))
            shas[ver] = m.group(1)
    op = dve_ops.DveOp(name, spec, subdim=False, uops_sha=shas)
    if not any(o.name == name for o in dve_ops.OPS):
        dve_ops.OPS.append(op)
    dve_ops.CUSTOM_DVE_SPECS[name] = spec
    _cache["sqdiff"] = op
    return op


def _band_c0(rt, w, n=N):
    """Rotated-space band window start for local row tile rt."""
    center = ROT_BASE + rt * 128 + 64
    return max(0, min(n - w, center - w // 2))


def _build_program(w, n=N, shard=SHARD, nb=B):
    """Emit the per-core Bass program. Identical on all cores (pure SPMD)."""
    import concourse.bacc as bacc
    import concourse.tile as tile
    from concourse import mybir

    f32 = mybir.dt.float32
    b16 = mybir.dt.bfloat16
    AX = mybir.AxisListType
    ACTF = mybir.ActivationFunctionType

    rts = shard // 128
    chunks = [(0, 1536), (1536, 3072), (3072, 4096)]
    bd_bufs = 2 if w <= 512 else 1  # psum: 6 banks for chunks + band

    nc = bacc.Bacc("TRN2", target_bir_lowering=False, debug=False,
                   num_devices=NCORES)

    RP = nc.dram_tensor("rp", [nb, KP, n], b16, kind="ExternalInput").ap()
    RF = nc.dram_tensor("rf", [nb, KF, n], b16, kind="ExternalInput").ap()
    # combined lhsT: cols [0:shard] = lf (128 rows), cols [shard:2*shard] =
    # lp (rows 0:KP, rest zero-padded) -> one DMA per batch
    LPF = nc.dram_tensor("lpf", [nb, 128, 2 * shard], b16,
                         kind="ExternalInput").ap()
    WV = nc.dram_tensor("wv", [128, nb * rts], f32, kind="ExternalInput").ap()
    OUT = nc.dram_tensor("out", [128, nb * rts], f32,
                         kind="ExternalOutput").ap()

    with tile.TileContext(nc) as tc:
        with (
            tc.tile_pool(name="rhs", bufs=2) as rhs_pool,
            tc.tile_pool(name="lhs", bufs=2) as lhs_pool,
            tc.tile_pool(name="efb", bufs=3) as ef_pool,
            tc.tile_pool(name="epb", bufs=3) as ep_pool,
            tc.tile_pool(name="ubuf", bufs=2) as u_pool,
            tc.tile_pool(name="small", bufs=8) as small,
            tc.tile_pool(name="acc", bufs=1) as acc_pool,
            tc.tile_pool(name="psA", bufs=2, space="PSUM") as psA,
            tc.tile_pool(name="psB", bufs=bd_bufs, space="PSUM") as psB,
        ):
            li_cols = acc_pool.tile([128, nb * rts], f32)
            w_all = acc_pool.tile([128, nb * rts], f32)

            # dummy exp to hoist the ACT table load off the critical path
            warm = small.tile([1, 1], f32, tag="warm")
            nc.vector.memset(warm, 0.0)
            nc.scalar.activation(out=warm, in_=warm, func=ACTF.Exp)

            # PE p-state warm-up: get the tensor engine off the cold clock
            # while the first DMAs are in flight
            pe_w = acc_pool.tile([1, 512], b16)
            nc.vector.memset(pe_w, 0.0)
            wcols = min(w, 512)
            for k in range(6):
                pwp = psB.tile([128, w], f32, tag="bd", name=f"pewarm{k}")
                nc.tensor.matmul(pwp[:, 0:wcols], pe_w[:, 0:128],
                                 pe_w[:, 0:wcols], start=True, stop=True)

            for b in range(nb):
                # lhsT + small band rhs first: they gate the first ACT inst
                lpf_t = lhs_pool.tile([128, 2 * shard], b16, tag="lpf")
                nc.sync.dma_start(out=lpf_t, in_=LPF[b])
                lf_t = lpf_t[:, 0:shard]
                lp_t = lpf_t[0:KP, shard:2 * shard]
                rp_t = rhs_pool.tile([KP, n], b16, tag="rp")
                nc.sync.dma_start(out=rp_t, in_=RP[b])
                # rf in 3 pieces matching the ACT chunks so chunk1 compute
                # starts as soon as its columns land
                rf_p = [rhs_pool.tile([KF, ce - cs], b16, tag=f"rf{pi}",
                                      name=f"rf_p{pi}")
                        for pi, (cs, ce) in enumerate(chunks)]
                for pi, (cs, ce) in enumerate(chunks):
                    nc.sync.dma_start(out=rf_p[pi], in_=RF[b][:, cs:ce])
                if b == 0:
                    # non-urgent; keep it off the critical first-RT DMA path
                    nc.sync.dma_start(out=w_all, in_=WV)

                for rt in range(rts):
                    r0 = rt * 128
                    c0 = _band_c0(rt, w, n)
                    col = b * rts + rt

                    e_f = ef_pool.tile([128, n], f32, tag="ef")
                    e_p = ep_pool.tile([128, n], f32, tag="ep")
                    sfc = small.tile([128, 4], f32, tag="sfc")
                    s_p = small.tile([128, 1], f32, tag="sp")

                    # zero the outside-band region of e_p on Pool
                    if c0 > 0:
                        nc.gpsimd.memset(e_p[:, 0:c0], 0.0)
                    if c0 + w < n:
                        nc.gpsimd.memset(e_p[:, c0 + w:n], 0.0)

                    # band: points matmul + exp (accum -> S_p)
                    pB = psB.tile([128, w], f32, tag="bd")
                    for q in range(0, w, 512):
                        qw = min(512, w - q)
                        nc.tensor.matmul(
                            pB[:, q:q + qw], lp_t[:, r0:r0 + 128],
                            rp_t[:, c0 + q:c0 + q + qw],
                            start=True, stop=True)
                    last_rt = (b == nb - 1 and rt == rts - 1)
                    if last_rt:
                        # keep S_p on the ACT accum for the final tile: the
                        # DVE is saturated at the tail and a reduce there
                        # would gate the last fused op
                        nc.scalar.activation(
                            out=e_p[:, c0:c0 + w], in_=pB, func=ACTF.Exp,
                            scale=1.0, accum_out=s_p)
                    else:
                        nc.scalar.activation(
                            out=e_p[:, c0:c0 + w], in_=pB, func=ACTF.Exp,
                            scale=1.0)
                        # S_p via DVE (saves the ACT accum-read; DVE has slack)
                        nc.vector.reduce_sum(s_p, e_p[:, c0:c0 + w],
                                             axis=AX.X)

                    # feature chunks: matmul + exp (accums -> sfc cols)
                    for ci, (cs, ce) in enumerate(chunks):
                        width = ce - cs
                        pA = psA.tile([128, 1536], f32, tag="A",
                                      name=f"A{ci}")
                        for q in range(0, width, 512):
                            nc.tensor.matmul(
                                pA[:, q:q + 512], lf_t[:, r0:r0 + 128],
                                rf_p[ci][:, q:q + 512],
                                start=True, stop=True)
                        nc.scalar.activation(
                            out=e_f[:, cs:ce], in_=pA[:, 0:width],
                            func=ACTF.Exp, scale=1.0,
                            accum_out=sfc[:, ci:ci + 1])

                    s_f = small.tile([128, 1], f32, tag="sf")
                    nc.vector.reduce_sum(s_f, sfc[:, 0:3], axis=AX.X)
                    rsf = small.tile([128, 1], f32, tag="rsf")
                    nc.vector.reciprocal(rsf, s_f)
                    rho = small.tile([128, 1], f32, tag="rho")
                    nc.vector.tensor_mul(rho, s_p, rsf)
                    rsp = small.tile([128, 1], f32, tag="rsp")
                    nc.vector.reciprocal(rsp, s_p)
                    rsp2 = small.tile([128, 1], f32, tag="rsp2")
                    nc.vector.tensor_mul(rsp2, rsp, rsp)
                    s1v = small.tile([128, 1], f32, tag="s1v")
                    nc.vector.tensor_mul(s1v, rsp2, w_all[:, col:col + 1])

                    u_t = u_pool.tile([128, n], b16, tag="u")
                    nc.vector._custom_dve(
                        _get_sqdiff_op(), out=u_t, in0=e_p, in1=e_f,
                        s0=rho, s1=s1v, accum_out=li_cols[:, col:col + 1])

            # ship the per-row-tile partials; host does the final sums
            nc.sync.dma_start(out=OUT, in_=li_cols)

    nc.compile()
    return nc


def _split(x, levels):
    """Split fp array into `levels` bf16 terms (hi, mid, lo...)."""
    parts = []
    r = np.asarray(x, np.float64)
    for _ in range(levels):
        h = r.astype(np.float32).astype(bf16)
        parts.append(h)
        r = r - h.astype(np.float64)
    return parts


def _rcm_order(pts):
    """Reverse Cuthill-McKee order of the significant-pair graph."""
    from scipy.sparse import csr_matrix
    from scipy.sparse.csgraph import reverse_cuthill_mckee
    xs = (pts / SIGMA).astype(np.float32)
    x2 = (xs * xs).sum(-1)
    s = 2.0 * (xs @ xs.T) - x2[None, :]  # score + |x_i|^2 (row shift ok)
    smax = s.max(1, keepdims=True)
    sig = s >= (smax - np.float32(RCM_DELTA))
    adj = csr_matrix(sig | sig.T)
    return np.asarray(reverse_cuthill_mckee(adj, symmetric_mode=True))


def _band_geometry(pts_sorted):
    """Per-row exp-mass column cumsums for window coverage analysis."""
    xs = (pts_sorted / SIGMA).astype(np.float32)
    x2 = (xs * xs).sum(-1)
    s = 2.0 * (xs @ xs.T) - x2[None, :] - x2[:, None]
    smax = s.max(1, keepdims=True)
    e = np.exp((s - smax).astype(np.float64))
    return np.cumsum(e, axis=1), e.sum(1)


def _tile_bound(csum, tot, w_sorted, t, lo, hi):
    """4*sum w_i*eps_i for tile t with (unclamped) global window [lo, hi)."""
    lo_c, hi_c = max(0, lo), min(N, hi)
    rows = slice(t * 128, (t + 1) * 128)
    if hi_c <= lo_c:
        cov = np.zeros(128)
    else:
        cov = csum[rows, hi_c - 1] - (csum[rows, lo_c - 1] if lo_c > 0 else 0.0)
    eps = np.maximum(1.0 - cov / tot[rows], 0.0)
    return float((w_sorted[rows] * 4.0 * eps).sum())


def _core_shift(csum, tot, w_sorted, w, c, shard=SHARD):
    """Best per-core window shift (windows move together) + its bound."""
    best_s, best_b = 0, np.inf
    for s in range(-128, 129, 8):
        bc = 0.0
        for rt in range(shard // 128):
            t = (c * shard) // 128 + rt
            lo = t * 128 + 64 - w // 2 - s
            bc += _tile_bound(csum, tot, w_sorted, t, lo, lo + w)
        if bc < best_b:
            best_s, best_b = s, bc
    return best_s, best_b


def _band_err_bound(pts_sorted, w_sorted, w):
    """Zero-shift bound (fallback path check)."""
    csum, tot = _band_geometry(pts_sorted)
    return sum(_tile_bound(csum, tot, w_sorted, t,
                           t * 128 + 64 - w // 2, t * 128 + 64 + w // 2)
               for t in range(N // 128))


def _prep_inputs(points, weights, pointfea1, pointfea2,
                 nb=B, n=N, shard=SHARD, ncores=NCORES):
    """Host-side sort/shard/layout prep -> (w, list of per-core input dicts)."""
    points = np.asarray(points, np.float64)
    weights = np.asarray(weights, np.float64)
    f1 = np.asarray(pointfea1, np.float64)
    f2 = np.asarray(pointfea2, np.float64)

    rts = shard // 128
    per_batch = []
    geos = []
    for b in range(nb):
        order = _rcm_order(points[b])
        p = points[b][order]
        per_batch.append((order, p, weights[b][order], f1[b][order],
                          f2[b][order]))
        geos.append(_band_geometry(p))

    # smallest window whose per-core-optimally-shifted coverage bound passes
    w_band, shifts = 1024, np.zeros((nb, ncores), np.int64)
    for w_try in (320, 352, 448):
        cand = np.zeros((nb, ncores), np.int64)
        bound = 0.0
        for b in range(nb):
            csum, tot = geos[b]
            for c in range(ncores):
                s, bc = _core_shift(csum, tot, per_batch[b][2], w_try, c)
                cand[b, c] = s
                bound += bc
        # summed (all batches/cores) dropped-mass bound; itself ~20x
        # conservative vs measured error, so 4e-3 keeps >5x real margin
        if bound < 4e-3:
            w_band, shifts = w_try, cand
            break
    if w_band > 512:
        for (order, p, w_, _, _) in per_batch:
            assert _band_err_bound(p, w_, w_band) < 5e-3, \
                "band window too narrow for data"

    rp_all = np.empty((nb, KP, n), bf16)
    rf_all = np.empty((nb, KF, n), bf16)
    lp_all = np.empty((nb, KP, n), bf16)   # full rows; sliced per core
    lf_all = np.empty((nb, KF, n), bf16)
    wv_all = np.empty((nb, n), np.float32)

    for b, (order, p, w_, a, c) in enumerate(per_batch):
        xs = p / SIGMA
        x2 = (xs * xs).sum(-1)
        xh, xm, xl = _split(xs, 3)
        y2h, y2m, y2l = _split(x2, 3)
        bph, bpm, bpl = _split(-x2, 3)

        for d in range(3):
            rrows = [2 * xh[..., d], 2 * xm[..., d], 2 * xh[..., d],
                     2 * xl[..., d], 2 * xh[..., d], 2 * xm[..., d]]
            lrows = [xh[..., d], xh[..., d], xm[..., d],
                     xh[..., d], xl[..., d], xm[..., d]]
            for k in range(6):
                rp_all[b, 6 * d + k] = rrows[k]
                lp_all[b, 6 * d + k] = lrows[k]
        rp_all[b, 18] = -y2h
        rp_all[b, 19] = -y2m
        rp_all[b, 20] = -y2l
        rp_all[b, 21:24] = np.float32(1.0)
        lp_all[b, 18:21] = np.float32(1.0)
        lp_all[b, 21] = bph
        lp_all[b, 22] = bpm
        lp_all[b, 23] = bpl

        g1 = (a * a).sum(-1)
        g2 = (c * c).sum(-1)
        f1h, f1l = _split(a, 2)
        f2h, _ = _split(c, 2)
        g2h, g2m = _split(g2, 2)
        bfh, bfm = _split(np.float64(SHIFT_F) - g1, 2)

        rf_all[b, :64] = (2 * f2h.astype(np.float32)).T
        rf_all[b, 64:64 + NLO] = (2 * f2h[:, :NLO].astype(np.float32)).T
        rf_all[b, 124] = -g2h
        rf_all[b, 125] = -g2m
        rf_all[b, 126:128] = np.float32(1.0)
        lf_all[b, :64] = f1h.T
        lf_all[b, 64:64 + NLO] = f1l[:, :NLO].T
        lf_all[b, 124:126] = np.float32(1.0)
        lf_all[b, 126] = bfh
        lf_all[b, 127] = bfm

        wv_all[b] = w_.astype(np.float32)

    in_maps = []
    for cidx in range(ncores):
        r0 = cidx * shard
        sl = slice(r0, r0 + shard)
        rp = np.empty_like(rp_all)
        rf = np.empty_like(rf_all)
        for b in range(nb):
            # per-(batch,core) roll: aligns this core's rows to the fixed
            # rotated-space windows, plus the data-optimized shift
            roll = ROT_BASE - r0 + int(shifts[b, cidx])
            rp[b] = np.roll(rp_all[b], roll, axis=1)
            rf[b] = np.roll(rf_all[b], roll, axis=1)
        rp = np.ascontiguousarray(rp)
        rf = np.ascontiguousarray(rf)
        lpf = np.zeros((nb, 128, 2 * shard), bf16)
        lpf[:, :, :shard] = lf_all[:, :, sl]
        lpf[:, :KP, shard:] = lp_all[:, :, sl]
        wvt = np.empty((128, nb * rts), np.float32)
        for b in range(nb):
            for rt in range(rts):
                wvt[:, b * rts + rt] = \
                    wv_all[b, r0 + rt * 128:r0 + (rt + 1) * 128]
        in_maps.append({"rp": rp, "rf": rf,
                        "lpf": np.ascontiguousarray(lpf), "wv": wvt})
    return w_band, in_maps


def kernel(points, weights, pointfea1, pointfea2):
    global _last_results
    from concourse.bass_utils import run_bass_kernel_spmd

    w_band, in_maps = _prep_inputs(points, weights, pointfea1, pointfea2)
    key = f"nc_w{w_band}"
    if key not in _cache:
        _cache[key] = _build_program(w_band)
        _cache["nc"] = _cache[key]  # for test harness TimelineSim
    nc = _cache[key]
    _cache["nc"] = nc

    res = run_bass_kernel_spmd(nc, in_maps, core_ids=list(range(NCORES)))
    _last_results = res
    rts = RT_PER_BATCH
    out = np.zeros(B, np.float64)
    for c in range(NCORES):
        li = np.asarray(res.results[c]["out"], np.float64)  # [128, B*rts]
        for b in range(B):
            out[b] += li[:, b * rts:(b + 1) * rts].sum()
    return out.astype(np.float32)


if __name__ == "__main__":
    rng = np.random.default_rng(0)
    pts = rng.random((B, N, 3), np.float32)
    w = rng.random((B, N), np.float32)
    w /= w.sum(1, keepdims=True)
    a = rng.standard_normal((B, N, D)).astype(np.float32)
    bfea = rng.standard_normal((B, N, D)).astype(np.float32)
    out = kernel(pts, w, a, bfea)
    print("kernel out:", out)


# revision 45
# speedup vs baseline: 1.0193x; 1.0024x over previous
"""TRN2 Bass kernel for nn_DeepFeatureLoss (B=4, N=4096, D=64, 8 cores).

Math (per batch b):
  P = softmax_j(-|x_i/s - x_j/s|^2)        (spatial gaussian, s=0.05)
  Q = softmax_j(-|f1_i - f2_j|^2)          (feature affinity)
  loss_b = sum_i w_i * sum_j (P_ij - Q_ij)^2

Key structural ideas (v2, banded points):
  - sigma=0.05 makes P extremely peaked: only spatially-near pairs matter.
    The host sorts each batch's points with reverse Cuthill-McKee on the
    significant-pair graph (score >= rowmax - 15), which empirically gives
    bandwidth ~220 on this data. After sorting, P is (nearly) banded.
  - Each core gets 512 rows; its column space is ROTATED (host-side roll)
    so its rows sit at rotated columns [256, 768). Band windows then become
    core-independent diagonal windows of width W=448 centred on each row
    tile, so one SPMD program serves all 8 cores. Column rotation is
    harmless: softmax row-sums are permutation-invariant. A quantitative
    dropped-mass bound selects W=1024 fallback if the data needs it.
  - e_p is computed only on the band (ACT exp of a [128, W] psum tile) and
    zero-extended into a full-width buffer (Pool memsets the outside), so
    the fused DVE op is identical to the full version:
       li = sum_j (e_p - rho*e_f)^2 * (w/S_p^2),  rho = S_p/S_f.
  - Features use ONE K=128 bf16 matmul pass (64 hi-product rows + 60
    lo-correction rows + 2 g2 rows + 2 bias rows); validated 1.7e-4 final
    rel err vs fp64 (tolerance 2e-2).
  - Per row tile ACT does 4 exp instructions (1536+1536+1024 feature chunks
    + W band) = 4096+W elems vs 8192 in the full version; PE does 4096+W
    cols vs 12288. S_p comes from a DVE reduce (ACT accum only on the final
    tile, where DVE saturation would put the reduce on the tail path).
"""

import os
import numpy as np
import ml_dtypes

bf16 = ml_dtypes.bfloat16

SIGMA = 0.05
SHIFT_F = 30.0  # lifts feature exp away from denormal S_f; validated on data
RCM_DELTA = 8.0  # significant-pair cutoff for the ordering graph; a tight
# graph makes RCM pack the truly-massive pairs into a narrower band

B, N, D = 4, 4096, 64
NCORES = 8
SHARD = N // NCORES          # 512 rows per core per batch
RT_PER_BATCH = SHARD // 128  # 4 row tiles of 128
ROT_BASE = 256               # core rows sit at rotated cols [256, 768)

KP = 24    # points: 18 coord-split rows + 3 y2 rows + 3 bias rows
KF = 128   # features: 64 hi + NLO lo-corrections + 2 g2 + 2 bias
NLO = 60

_cache = {}

_last_results = None  # stashed BassKernelResults for test harnesses


def _get_sqdiff_op():
    """Register (once) a fused DVE op:
        out = (in0 - in1*s0)^2 * s1 ; accum_out = sum_k out[k]
    i.e. li = sum_j (e_p - rho*e_f)^2 * (w/S_p^2) in ONE 1x DVE pass.
    """
    if "sqdiff" in _cache:
        return _cache["sqdiff"]
    import re
    from operator import add as _add
    from concourse import dve_ops
    from concourse.dve_spec import Spec, Src0, Src1, C0, C1, Zero, sq

    name = "SQDIFF_SCALE_RED_DFL"

    def _ref(in0, in1, s0, s1, imm2):
        b = (((in0.astype(np.float32) - in1 * s0) ** 2) * s1).astype(np.float32)
        return b, b.reshape(b.shape[0], -1).sum(axis=-1, keepdims=True)

    spec = Spec(body=sq(Src0 - Src1 * C0) * C1, accum=_add, accum_init=Zero,
                reference=_ref)
    if name not in dve_ops._SUB_OPCODE_FOR_NAME:
        row = max(dve_ops._SUB_OPCODE_FOR_NAME.values()) + 1
        assert row < 0x20
        dve_ops._SUB_OPCODE_FOR_NAME[name] = row
    shas = {}
    for ver in ("v3", "v4"):
        probe = dve_ops.DveOp(name, spec, subdim=False, uops_sha={})
        try:
            probe.compile(ver)
        except ValueError as e:
            m = re.search(r"\{ver\}.*?=\"([0-9a-f]+)\"".replace("{ver}", ver),
                          str(e)) or re.search(r'"([0-9a-f]{16})"', str(e))
            shas[ver] = m.group(1)
    op = dve_ops.DveOp(name, spec, subdim=False, uops_sha=shas)
    if not any(o.name == name for o in dve_ops.OPS):
        dve_ops.OPS.append(op)
    dve_ops.CUSTOM_DVE_SPECS[name] = spec
    _cache["sqdiff"] = op
    return op


def _band_c0(rt, w, n=N):
    """Rotated-space band window start for local row tile rt."""
    center = ROT_BASE + rt * 128 + 64
    return max(0, min(n - w, center - w // 2))


def _build_program(w, n=N, shard=SHARD, nb=B):
    """Emit the per-core Bass program. Identical on all cores (pure SPMD)."""
    import concourse.bacc as bacc
    import concourse.tile as tile
    from concourse import mybir

    f32 = mybir.dt.float32
    b16 = mybir.dt.bfloat16
    AX = mybir.AxisListType
    ACTF = mybir.ActivationFunctionType

    rts = shard // 128
    # first chunk smallest: its rhs DMA + matmuls gate the pipeline start
    chunks = [(0, 1024), (1024, 2560), (2560, 4096)]
    bd_bufs = 2 if w <= 512 else 1  # psum: 6 banks for chunks + band

    nc = bacc.Bacc("TRN2", target_bir_lowering=False, debug=False,
                   num_devices=NCORES)

    RP = nc.dram_tensor("rp", [nb, KP, n], b16, kind="ExternalInput").ap()
    RF = nc.dram_tensor("rf", [nb, KF, n], b16, kind="ExternalInput").ap()
    # combined lhsT: cols [0:shard] = lf (128 rows), cols [shard:2*shard] =
    # lp (rows 0:KP, rest zero-padded) -> one DMA per batch
    LPF = nc.dram_tensor("lpf", [nb, 128, 2 * shard], b16,
                         kind="ExternalInput").ap()
    WV = nc.dram_tensor("wv", [128, nb * rts], f32, kind="ExternalInput").ap()
    OUT = nc.dram_tensor("out", [128, nb * rts], f32,
                         kind="ExternalOutput").ap()

    with tile.TileContext(nc) as tc:
        with (
            tc.tile_pool(name="rhs", bufs=2) as rhs_pool,
            tc.tile_pool(name="lhs", bufs=2) as lhs_pool,
            tc.tile_pool(name="efb", bufs=3) as ef_pool,
            tc.tile_pool(name="epb", bufs=3) as ep_pool,
            tc.tile_pool(name="ubuf", bufs=2) as u_pool,
            tc.tile_pool(name="small", bufs=8) as small,
            tc.tile_pool(name="acc", bufs=1) as acc_pool,
            tc.tile_pool(name="psA", bufs=2, space="PSUM") as psA,
            tc.tile_pool(name="psB", bufs=bd_bufs, space="PSUM") as psB,
        ):
            li_cols = acc_pool.tile([128, nb * rts], f32)
            w_all = acc_pool.tile([128, nb * rts], f32)

            # dummy exp to hoist the ACT table load off the critical path
            warm = small.tile([1, 1], f32, tag="warm")
            nc.vector.memset(warm, 0.0)
            nc.scalar.activation(out=warm, in_=warm, func=ACTF.Exp)

            # PE p-state warm-up: get the tensor engine off the cold clock
            # while the first DMAs are in flight
            pe_w = acc_pool.tile([1, 512], b16)
            nc.vector.memset(pe_w, 0.0)
            wcols = min(w, 512)
            for k in range(6):
                pwp = psB.tile([128, w], f32, tag="bd", name=f"pewarm{k}")
                nc.tensor.matmul(pwp[:, 0:wcols], pe_w[:, 0:128],
                                 pe_w[:, 0:wcols], start=True, stop=True)

            for b in range(nb):
                # lhsT + small band rhs first: they gate the first ACT inst
                lpf_t = lhs_pool.tile([128, 2 * shard], b16, tag="lpf")
                nc.sync.dma_start(out=lpf_t, in_=LPF[b])
                lf_t = lpf_t[:, 0:shard]
                lp_t = lpf_t[0:KP, shard:2 * shard]
                rp_t = rhs_pool.tile([KP, n], b16, tag="rp")
                nc.sync.dma_start(out=rp_t, in_=RP[b])
                # rf in 3 pieces matching the ACT chunks so chunk1 compute
                # starts as soon as its columns land
                rf_p = [rhs_pool.tile([KF, ce - cs], b16, tag=f"rf{pi}",
                                      name=f"rf_p{pi}")
                        for pi, (cs, ce) in enumerate(chunks)]
                for pi, (cs, ce) in enumerate(chunks):
                    nc.sync.dma_start(out=rf_p[pi], in_=RF[b][:, cs:ce])

                def rf_cols(j0):
                    """(piece tile, offset) holding columns [j0, j0+512)."""
                    for pi, (ps, pe_) in enumerate(chunks):
                        if ps <= j0 < pe_:
                            return rf_p[pi], j0 - ps
                    raise AssertionError(j0)
                if b == 0:
                    # non-urgent; keep it off the critical first-RT DMA path
                    nc.sync.dma_start(out=w_all, in_=WV)

                for rt in range(rts):
                    r0 = rt * 128
                    c0 = _band_c0(rt, w, n)
                    col = b * rts + rt

                    e_f = ef_pool.tile([128, n], f32, tag="ef")
                    e_p = ep_pool.tile([128, n], f32, tag="ep")
                    sfc = small.tile([128, 4], f32, tag="sfc")
                    s_p = small.tile([128, 1], f32, tag="sp")

                    # zero the outside-band region of e_p on Pool
                    if c0 > 0:
                        nc.gpsimd.memset(e_p[:, 0:c0], 0.0)
                    if c0 + w < n:
                        nc.gpsimd.memset(e_p[:, c0 + w:n], 0.0)

                    # band: points matmul + exp (accum -> S_p)
                    pB = psB.tile([128, w], f32, tag="bd")
                    for q in range(0, w, 512):
                        qw = min(512, w - q)
                        nc.tensor.matmul(
                            pB[:, q:q + qw], lp_t[:, r0:r0 + 128],
                            rp_t[:, c0 + q:c0 + q + qw],
                            start=True, stop=True)
                    last_rt = (b == nb - 1 and rt == rts - 1)
                    if last_rt:
                        # keep S_p on the ACT accum for the final tile: the
                        # DVE is saturated at the tail and a reduce there
                        # would gate the last fused op
                        nc.scalar.activation(
                            out=e_p[:, c0:c0 + w], in_=pB, func=ACTF.Exp,
                            scale=1.0, accum_out=s_p)
                    else:
                        nc.scalar.activation(
                            out=e_p[:, c0:c0 + w], in_=pB, func=ACTF.Exp,
                            scale=1.0)
                        # S_p via DVE (saves the ACT accum-read; DVE has slack)
                        nc.vector.reduce_sum(s_p, e_p[:, c0:c0 + w],
                                             axis=AX.X)

                    # feature chunks: matmul + exp (accums -> sfc cols)
                    for ci, (cs, ce) in enumerate(chunks):
                        width = ce - cs
                        pA = psA.tile([128, 1536], f32, tag="A",
                                      name=f"A{ci}")
                        for q in range(0, width, 512):
                            rtile, off = rf_cols(cs + q)
                            nc.tensor.matmul(
                                pA[:, q:q + 512], lf_t[:, r0:r0 + 128],
                                rtile[:, off:off + 512],
                                start=True, stop=True)
                        nc.scalar.activation(
                            out=e_f[:, cs:ce], in_=pA[:, 0:width],
                            func=ACTF.Exp, scale=1.0,
                            accum_out=sfc[:, ci:ci + 1])

                    s_f = small.tile([128, 1], f32, tag="sf")
                    nc.vector.reduce_sum(s_f, sfc[:, 0:3], axis=AX.X)
                    rsf = small.tile([128, 1], f32, tag="rsf")
                    nc.vector.reciprocal(rsf, s_f)
                    rho = small.tile([128, 1], f32, tag="rho")
                    nc.vector.tensor_mul(rho, s_p, rsf)
                    rsp = small.tile([128, 1], f32, tag="rsp")
                    nc.vector.reciprocal(rsp, s_p)
                    rsp2 = small.tile([128, 1], f32, tag="rsp2")
                    nc.vector.tensor_mul(rsp2, rsp, rsp)
                    s1v = small.tile([128, 1], f32, tag="s1v")
                    nc.vector.tensor_mul(s1v, rsp2, w_all[:, col:col + 1])

                    u_t = u_pool.tile([128, n], b16, tag="u")
                    nc.vector._custom_dve(
                        _get_sqdiff_op(), out=u_t, in0=e_p, in1=e_f,
                        s0=rho, s1=s1v, accum_out=li_cols[:, col:col + 1])

            # ship the per-row-tile partials; host does the final sums
            nc.sync.dma_start(out=OUT, in_=li_cols)

    nc.compile()
    return nc


def _split(x, levels):
    """Split fp array into `levels` bf16 terms (hi, mid, lo...)."""
    parts = []
    r = np.asarray(x, np.float64)
    for _ in range(levels):
        h = r.astype(np.float32).astype(bf16)
        parts.append(h)
        r = r - h.astype(np.float64)
    return parts


def _rcm_order(pts):
    """Reverse Cuthill-McKee order of the significant-pair graph."""
    from scipy.sparse import csr_matrix
    from scipy.sparse.csgraph import reverse_cuthill_mckee
    xs = (pts / SIGMA).astype(np.float32)
    x2 = (xs * xs).sum(-1)
    s = 2.0 * (xs @ xs.T) - x2[None, :]  # score + |x_i|^2 (row shift ok)
    smax = s.max(1, keepdims=True)
    sig = s >= (smax - np.float32(RCM_DELTA))
    adj = csr_matrix(sig | sig.T)
    return np.asarray(reverse_cuthill_mckee(adj, symmetric_mode=True))


def _band_geometry(pts_sorted):
    """Per-row exp-mass column cumsums for window coverage analysis."""
    xs = (pts_sorted / SIGMA).astype(np.float32)
    x2 = (xs * xs).sum(-1)
    s = 2.0 * (xs @ xs.T) - x2[None, :] - x2[:, None]
    smax = s.max(1, keepdims=True)
    e = np.exp((s - smax).astype(np.float64))
    return np.cumsum(e, axis=1), e.sum(1)


def _tile_bound(csum, tot, w_sorted, t, lo, hi):
    """4*sum w_i*eps_i for tile t with (unclamped) global window [lo, hi)."""
    lo_c, hi_c = max(0, lo), min(N, hi)
    rows = slice(t * 128, (t + 1) * 128)
    if hi_c <= lo_c:
        cov = np.zeros(128)
    else:
        cov = csum[rows, hi_c - 1] - (csum[rows, lo_c - 1] if lo_c > 0 else 0.0)
    eps = np.maximum(1.0 - cov / tot[rows], 0.0)
    return float((w_sorted[rows] * 4.0 * eps).sum())


def _core_shift(csum, tot, w_sorted, w, c, shard=SHARD):
    """Best per-core window shift (windows move together) + its bound."""
    best_s, best_b = 0, np.inf
    for s in range(-128, 129, 8):
        bc = 0.0
        for rt in range(shard // 128):
            t = (c * shard) // 128 + rt
            lo = t * 128 + 64 - w // 2 - s
            bc += _tile_bound(csum, tot, w_sorted, t, lo, lo + w)
        if bc < best_b:
            best_s, best_b = s, bc
    return best_s, best_b


def _band_err_bound(pts_sorted, w_sorted, w):
    """Zero-shift bound (fallback path check)."""
    csum, tot = _band_geometry(pts_sorted)
    return sum(_tile_bound(csum, tot, w_sorted, t,
                           t * 128 + 64 - w // 2, t * 128 + 64 + w // 2)
               for t in range(N // 128))


def _prep_inputs(points, weights, pointfea1, pointfea2,
                 nb=B, n=N, shard=SHARD, ncores=NCORES):
    """Host-side sort/shard/layout prep -> (w, list of per-core input dicts)."""
    points = np.asarray(points, np.float64)
    weights = np.asarray(weights, np.float64)
    f1 = np.asarray(pointfea1, np.float64)
    f2 = np.asarray(pointfea2, np.float64)

    rts = shard // 128
    per_batch = []
    geos = []
    for b in range(nb):
        order = _rcm_order(points[b])
        p = points[b][order]
        per_batch.append((order, p, weights[b][order], f1[b][order],
                          f2[b][order]))
        geos.append(_band_geometry(p))

    # smallest window whose per-core-optimally-shifted coverage bound passes
    w_band, shifts = 1024, np.zeros((nb, ncores), np.int64)
    for w_try in (320, 352, 448):
        cand = np.zeros((nb, ncores), np.int64)
        bound = 0.0
        for b in range(nb):
            csum, tot = geos[b]
            for c in range(ncores):
                s, bc = _core_shift(csum, tot, per_batch[b][2], w_try, c)
                cand[b, c] = s
                bound += bc
        # summed (all batches/cores) dropped-mass bound; itself ~20x
        # conservative vs measured error, so 4e-3 keeps >5x real margin
        if bound < 4e-3:
            w_band, shifts = w_try, cand
            break
    if w_band > 512:
        for (order, p, w_, _, _) in per_batch:
            assert _band_err_bound(p, w_, w_band) < 5e-3, \
                "band window too narrow for data"

    rp_all = np.empty((nb, KP, n), bf16)
    rf_all = np.empty((nb, KF, n), bf16)
    lp_all = np.empty((nb, KP, n), bf16)   # full rows; sliced per core
    lf_all = np.empty((nb, KF, n), bf16)
    wv_all = np.empty((nb, n), np.float32)

    for b, (order, p, w_, a, c) in enumerate(per_batch):
        xs = p / SIGMA
        x2 = (xs * xs).sum(-1)
        xh, xm, xl = _split(xs, 3)
        y2h, y2m, y2l = _split(x2, 3)
        bph, bpm, bpl = _split(-x2, 3)

        for d in range(3):
            rrows = [2 * xh[..., d], 2 * xm[..., d], 2 * xh[..., d],
                     2 * xl[..., d], 2 * xh[..., d], 2 * xm[..., d]]
            lrows = [xh[..., d], xh[..., d], xm[..., d],
                     xh[..., d], xl[..., d], xm[..., d]]
            for k in range(6):
                rp_all[b, 6 * d + k] = rrows[k]
                lp_all[b, 6 * d + k] = lrows[k]
        rp_all[b, 18] = -y2h
        rp_all[b, 19] = -y2m
        rp_all[b, 20] = -y2l
        rp_all[b, 21:24] = np.float32(1.0)
        lp_all[b, 18:21] = np.float32(1.0)
        lp_all[b, 21] = bph
        lp_all[b, 22] = bpm
        lp_all[b, 23] = bpl

        g1 = (a * a).sum(-1)
        g2 = (c * c).sum(-1)
        f1h, f1l = _split(a, 2)
        f2h, _ = _split(c, 2)
        g2h, g2m = _split(g2, 2)
        bfh, bfm = _split(np.float64(SHIFT_F) - g1, 2)

        rf_all[b, :64] = (2 * f2h.astype(np.float32)).T
        rf_all[b, 64:64 + NLO] = (2 * f2h[:, :NLO].astype(np.float32)).T
        rf_all[b, 124] = -g2h
        rf_all[b, 125] = -g2m
        rf_all[b, 126:128] = np.float32(1.0)
        lf_all[b, :64] = f1h.T
        lf_all[b, 64:64 + NLO] = f1l[:, :NLO].T
        lf_all[b, 124:126] = np.float32(1.0)
        lf_all[b, 126] = bfh
        lf_all[b, 127] = bfm

        wv_all[b] = w_.astype(np.float32)

    in_maps = []
    for cidx in range(ncores):
        r0 = cidx * shard
        sl = slice(r0, r0 + shard)
        rp = np.empty_like(rp_all)
        rf = np.empty_like(rf_all)
        for b in range(nb):
            # per-(batch,core) roll: aligns this core's rows to the fixed
            # rotated-space windows, plus the data-optimized shift
            roll = ROT_BASE - r0 + int(shifts[b, cidx])
            rp[b] = np.roll(rp_all[b], roll, axis=1)
            rf[b] = np.roll(rf_all[b], roll, axis=1)
        rp = np.ascontiguousarray(rp)
        rf = np.ascontiguousarray(rf)
        lpf = np.zeros((nb, 128, 2 * shard), bf16)
        lpf[:, :, :shard] = lf_all[:, :, sl]
        lpf[:, :KP, shard:] = lp_all[:, :, sl]
        wvt = np.empty((128, nb * rts), np.float32)
        for b in range(nb):
            for rt in range(rts):
                wvt[:, b * rts + rt] = \
                    wv_all[b, r0 + rt * 128:r0 + (rt + 1) * 128]
        in_maps.append({"rp": rp, "rf": rf,
                        "lpf": np.ascontiguousarray(lpf), "wv": wvt})
    return w_band, in_maps


def kernel(points, weights, pointfea1, pointfea2):
    global _last_results
    from concourse.bass_utils import run_bass_kernel_spmd

    w_band, in_maps = _prep_inputs(points, weights, pointfea1, pointfea2)
    key = f"nc_w{w_band}"
    if key not in _cache:
        _cache[key] = _build_program(w_band)
        _cache["nc"] = _cache[key]  # for test harness TimelineSim
    nc = _cache[key]
    _cache["nc"] = nc

    res = run_bass_kernel_spmd(nc, in_maps, core_ids=list(range(NCORES)))
    _last_results = res
    rts = RT_PER_BATCH
    out = np.zeros(B, np.float64)
    for c in range(NCORES):
        li = np.asarray(res.results[c]["out"], np.float64)  # [128, B*rts]
        for b in range(B):
            out[b] += li[:, b * rts:(b + 1) * rts].sum()
    return out.astype(np.float32)


if __name__ == "__main__":
    rng = np.random.default_rng(0)
    pts = rng.random((B, N, 3), np.float32)
    w = rng.random((B, N), np.float32)
    w /= w.sum(1, keepdims=True)
    a = rng.standard_normal((B, N, D)).astype(np.float32)
    bfea = rng.standard_normal((B, N, D)).astype(np.float32)
    out = kernel(pts, w, a, bfea)
    print("kernel out:", out)


# revision 46
# speedup vs baseline: 1.0225x; 1.0031x over previous
"""TRN2 Bass kernel for nn_DeepFeatureLoss (B=4, N=4096, D=64, 8 cores).

Math (per batch b):
  P = softmax_j(-|x_i/s - x_j/s|^2)        (spatial gaussian, s=0.05)
  Q = softmax_j(-|f1_i - f2_j|^2)          (feature affinity)
  loss_b = sum_i w_i * sum_j (P_ij - Q_ij)^2

Key structural ideas (v2, banded points):
  - sigma=0.05 makes P extremely peaked: only spatially-near pairs matter.
    The host sorts each batch's points with reverse Cuthill-McKee on the
    significant-pair graph (score >= rowmax - 15), which empirically gives
    bandwidth ~220 on this data. After sorting, P is (nearly) banded.
  - Each core gets 512 rows; its column space is ROTATED (host-side roll)
    so its rows sit at rotated columns [256, 768). Band windows then become
    core-independent diagonal windows of width W=448 centred on each row
    tile, so one SPMD program serves all 8 cores. Column rotation is
    harmless: softmax row-sums are permutation-invariant. A quantitative
    dropped-mass bound selects W=1024 fallback if the data needs it.
  - e_p is computed only on the band (ACT exp of a [128, W] psum tile) and
    zero-extended into a full-width buffer (Pool memsets the outside), so
    the fused DVE op is identical to the full version:
       li = sum_j (e_p - rho*e_f)^2 * (w/S_p^2),  rho = S_p/S_f.
  - Features use ONE K=128 bf16 matmul pass (64 hi-product rows + 60
    lo-correction rows + 2 g2 rows + 2 bias rows); validated 1.7e-4 final
    rel err vs fp64 (tolerance 2e-2).
  - Per row tile ACT does 4 exp instructions (1536+1536+1024 feature chunks
    + W band) = 4096+W elems vs 8192 in the full version; PE does 4096+W
    cols vs 12288. S_p comes from a DVE reduce (ACT accum only on the final
    tile, where DVE saturation would put the reduce on the tail path).
"""

import os
import numpy as np
import ml_dtypes

bf16 = ml_dtypes.bfloat16

SIGMA = 0.05
SHIFT_F = 30.0  # lifts feature exp away from denormal S_f; validated on data
RCM_DELTA = 8.0  # significant-pair cutoff for the ordering graph; a tight
# graph makes RCM pack the truly-massive pairs into a narrower band

B, N, D = 4, 4096, 64
NCORES = 8
SHARD = N // NCORES          # 512 rows per core per batch
RT_PER_BATCH = SHARD // 128  # 4 row tiles of 128
ROT_BASE = 256               # core rows sit at rotated cols [256, 768)

KP = 24    # points: 18 coord-split rows + 3 y2 rows + 3 bias rows
KF = 128   # features: 64 hi + NLO lo-corrections + 2 g2 + 2 bias
NLO = 60

_cache = {}

_last_results = None  # stashed BassKernelResults for test harnesses


def _get_sqdiff_op():
    """Register (once) a fused DVE op:
        out = (in0 - in1*s0)^2 * s1 ; accum_out = sum_k out[k]
    i.e. li = sum_j (e_p - rho*e_f)^2 * (w/S_p^2) in ONE 1x DVE pass.
    """
    if "sqdiff" in _cache:
        return _cache["sqdiff"]
    import re
    from operator import add as _add
    from concourse import dve_ops
    from concourse.dve_spec import Spec, Src0, Src1, C0, C1, Zero, sq

    name = "SQDIFF_SCALE_RED_DFL"

    def _ref(in0, in1, s0, s1, imm2):
        b = (((in0.astype(np.float32) - in1 * s0) ** 2) * s1).astype(np.float32)
        return b, b.reshape(b.shape[0], -1).sum(axis=-1, keepdims=True)

    spec = Spec(body=sq(Src0 - Src1 * C0) * C1, accum=_add, accum_init=Zero,
                reference=_ref)
    if name not in dve_ops._SUB_OPCODE_FOR_NAME:
        row = max(dve_ops._SUB_OPCODE_FOR_NAME.values()) + 1
        assert row < 0x20
        dve_ops._SUB_OPCODE_FOR_NAME[name] = row
    shas = {}
    for ver in ("v3", "v4"):
        probe = dve_ops.DveOp(name, spec, subdim=False, uops_sha={})
        try:
            probe.compile(ver)
        except ValueError as e:
            m = re.search(r"\{ver\}.*?=\"([0-9a-f]+)\"".replace("{ver}", ver),
                          str(e)) or re.search(r'"([0-9a-f]{16})"', str(e))
            shas[ver] = m.group(1)
    op = dve_ops.DveOp(name, spec, subdim=False, uops_sha=shas)
    if not any(o.name == name for o in dve_ops.OPS):
        dve_ops.OPS.append(op)
    dve_ops.CUSTOM_DVE_SPECS[name] = spec
    _cache["sqdiff"] = op
    return op


def _band_c0(rt, w, n=N):
    """Rotated-space band window start for local row tile rt."""
    center = ROT_BASE + rt * 128 + 64
    return max(0, min(n - w, center - w // 2))


def _build_program(w, n=N, shard=SHARD, nb=B):
    """Emit the per-core Bass program. Identical on all cores (pure SPMD)."""
    import concourse.bacc as bacc
    import concourse.tile as tile
    from concourse import mybir

    f32 = mybir.dt.float32
    b16 = mybir.dt.bfloat16
    AX = mybir.AxisListType
    ACTF = mybir.ActivationFunctionType

    rts = shard // 128
    # first chunk smallest: its rhs DMA + matmuls gate the pipeline start
    chunks = [(0, 1024), (1024, 2560), (2560, 4096)]
    bd_bufs = 2 if w <= 512 else 1  # psum: 6 banks for chunks + band

    nc = bacc.Bacc("TRN2", target_bir_lowering=False, debug=False,
                   num_devices=NCORES)

    RP = nc.dram_tensor("rp", [nb, KP, n], b16, kind="ExternalInput").ap()
    RF = nc.dram_tensor("rf", [nb, KF, n], b16, kind="ExternalInput").ap()
    # combined lhsT: cols [0:shard] = lf (128 rows), cols [shard:2*shard] =
    # lp (rows 0:KP, rest zero-padded) -> one DMA per batch
    LPF = nc.dram_tensor("lpf", [nb, 128, 2 * shard], b16,
                         kind="ExternalInput").ap()
    WV = nc.dram_tensor("wv", [128, nb * rts], f32, kind="ExternalInput").ap()
    OUT = nc.dram_tensor("out", [128, nb * rts], f32,
                         kind="ExternalOutput").ap()

    with tile.TileContext(nc) as tc:
        with (
            tc.tile_pool(name="rhs", bufs=2) as rhs_pool,
            tc.tile_pool(name="lhs", bufs=2) as lhs_pool,
            tc.tile_pool(name="efb", bufs=3) as ef_pool,
            tc.tile_pool(name="epb", bufs=3) as ep_pool,
            tc.tile_pool(name="ubuf", bufs=2) as u_pool,
            tc.tile_pool(name="small", bufs=8) as small,
            tc.tile_pool(name="acc", bufs=1) as acc_pool,
            tc.tile_pool(name="psA", bufs=2, space="PSUM") as psA,
            tc.tile_pool(name="psB", bufs=bd_bufs, space="PSUM") as psB,
        ):
            li_cols = acc_pool.tile([128, nb * rts], f32)
            w_all = acc_pool.tile([128, nb * rts], f32)

            # dummy exp to hoist the ACT table load off the critical path
            warm = small.tile([1, 1], f32, tag="warm")
            nc.vector.memset(warm, 0.0)
            nc.scalar.activation(out=warm, in_=warm, func=ACTF.Exp)

            # PE p-state warm-up: get the tensor engine off the cold clock
            # while the first DMAs are in flight
            pe_w = acc_pool.tile([1, 512], b16)
            nc.vector.memset(pe_w, 0.0)
            wcols = min(w, 512)
            for k in range(6):
                pwp = psB.tile([128, w], f32, tag="bd", name=f"pewarm{k}")
                nc.tensor.matmul(pwp[:, 0:wcols], pe_w[:, 0:128],
                                 pe_w[:, 0:wcols], start=True, stop=True)

            for b in range(nb):
                # lhsT + small band rhs first: they gate the first ACT inst
                lpf_t = lhs_pool.tile([128, 2 * shard], b16, tag="lpf")
                nc.sync.dma_start(out=lpf_t, in_=LPF[b])
                lf_t = lpf_t[:, 0:shard]
                lp_t = lpf_t[0:KP, shard:2 * shard]
                rp_t = rhs_pool.tile([KP, n], b16, tag="rp")
                nc.sync.dma_start(out=rp_t, in_=RP[b])
                # rf in 3 pieces matching the ACT chunks so chunk1 compute
                # starts as soon as its columns land
                rf_p = [rhs_pool.tile([KF, ce - cs], b16, tag=f"rf{pi}",
                                      name=f"rf_p{pi}")
                        for pi, (cs, ce) in enumerate(chunks)]
                for pi, (cs, ce) in enumerate(chunks):
                    nc.sync.dma_start(out=rf_p[pi], in_=RF[b][:, cs:ce])

                def rf_cols(j0):
                    """(piece tile, offset) holding columns [j0, j0+512)."""
                    for pi, (ps, pe_) in enumerate(chunks):
                        if ps <= j0 < pe_:
                            return rf_p[pi], j0 - ps
                    raise AssertionError(j0)
                if b == 0:
                    # non-urgent; keep it off the critical first-RT DMA path
                    nc.sync.dma_start(out=w_all, in_=WV)

                for rt in range(rts):
                    r0 = rt * 128
                    c0 = _band_c0(rt, w, n)
                    col = b * rts + rt

                    e_f = ef_pool.tile([128, n], f32, tag="ef")
                    e_p = ep_pool.tile([128, n], f32, tag="ep")
                    sfc = small.tile([128, 4], f32, tag="sfc")
                    s_p = small.tile([128, 1], f32, tag="sp")

                    # zero the outside-band region of e_p on Pool
                    if c0 > 0:
                        nc.gpsimd.memset(e_p[:, 0:c0], 0.0)
                    if c0 + w < n:
                        nc.gpsimd.memset(e_p[:, c0 + w:n], 0.0)

                    # band: points matmul + exp (accum -> S_p)
                    pB = psB.tile([128, w], f32, tag="bd")
                    for q in range(0, w, 512):
                        qw = min(512, w - q)
                        nc.tensor.matmul(
                            pB[:, q:q + qw], lp_t[:, r0:r0 + 128],
                            rp_t[:, c0 + q:c0 + q + qw],
                            start=True, stop=True)
                    last_rt = (b == nb - 1 and rt == rts - 1)
                    if last_rt:
                        # keep S_p on the ACT accum for the final tile: the
                        # DVE is saturated at the tail and a reduce there
                        # would gate the last fused op
                        nc.scalar.activation(
                            out=e_p[:, c0:c0 + w], in_=pB, func=ACTF.Exp,
                            scale=1.0, accum_out=s_p)
                    else:
                        nc.scalar.activation(
                            out=e_p[:, c0:c0 + w], in_=pB, func=ACTF.Exp,
                            scale=1.0)
                        # S_p via DVE (saves the ACT accum-read; DVE has slack)
                        nc.vector.reduce_sum(s_p, e_p[:, c0:c0 + w],
                                             axis=AX.X)

                    # feature chunks: matmul + exp (accums -> sfc cols)
                    for ci, (cs, ce) in enumerate(chunks):
                        width = ce - cs
                        pA = psA.tile([128, 1536], f32, tag="A",
                                      name=f"A{ci}")
                        for q in range(0, width, 512):
                            rtile, off = rf_cols(cs + q)
                            nc.tensor.matmul(
                                pA[:, q:q + 512], lf_t[:, r0:r0 + 128],
                                rtile[:, off:off + 512],
                                start=True, stop=True)
                        nc.scalar.activation(
                            out=e_f[:, cs:ce], in_=pA[:, 0:width],
                            func=ACTF.Exp, scale=1.0,
                            accum_out=sfc[:, ci:ci + 1])

                    s_f = small.tile([128, 1], f32, tag="sf")
                    nc.vector.reduce_sum(s_f, sfc[:, 0:3], axis=AX.X)
                    rsf = small.tile([128, 1], f32, tag="rsf")
                    nc.vector.reciprocal(rsf, s_f)
                    rho = small.tile([128, 1], f32, tag="rho")
                    nc.vector.tensor_mul(rho, s_p, rsf)
                    rsp = small.tile([128, 1], f32, tag="rsp")
                    nc.vector.reciprocal(rsp, s_p)
                    rsp2 = small.tile([128, 1], f32, tag="rsp2")
                    nc.vector.tensor_mul(rsp2, rsp, rsp)
                    s1v = small.tile([128, 1], f32, tag="s1v")
                    nc.vector.tensor_mul(s1v, rsp2, w_all[:, col:col + 1])

                    u_t = u_pool.tile([128, n], b16, tag="u")
                    nc.vector._custom_dve(
                        _get_sqdiff_op(), out=u_t, in0=e_p, in1=e_f,
                        s0=rho, s1=s1v, accum_out=li_cols[:, col:col + 1])

            # ship the per-row-tile partials; host does the final sums
            nc.sync.dma_start(out=OUT, in_=li_cols)

    nc.compile()
    return nc


def _split(x, levels):
    """Split fp array into `levels` bf16 terms (hi, mid, lo...)."""
    parts = []
    r = np.asarray(x, np.float64)
    for _ in range(levels):
        h = r.astype(np.float32).astype(bf16)
        parts.append(h)
        r = r - h.astype(np.float64)
    return parts


def _rcm_order(pts):
    """Reverse Cuthill-McKee order of the significant-pair graph."""
    from scipy.sparse import csr_matrix
    from scipy.sparse.csgraph import reverse_cuthill_mckee
    xs = (pts / SIGMA).astype(np.float32)
    x2 = (xs * xs).sum(-1)
    s = 2.0 * (xs @ xs.T) - x2[None, :]  # score + |x_i|^2 (row shift ok)
    smax = s.max(1, keepdims=True)
    sig = s >= (smax - np.float32(RCM_DELTA))
    adj = csr_matrix(sig | sig.T)
    return np.asarray(reverse_cuthill_mckee(adj, symmetric_mode=True))


def _band_geometry(pts_sorted):
    """Per-row exp-mass column cumsums for window coverage analysis."""
    xs = (pts_sorted / SIGMA).astype(np.float32)
    x2 = (xs * xs).sum(-1)
    s = 2.0 * (xs @ xs.T) - x2[None, :] - x2[:, None]
    smax = s.max(1, keepdims=True)
    e = np.exp((s - smax).astype(np.float64))
    return np.cumsum(e, axis=1), e.sum(1)


def _tile_bound(csum, tot, w_sorted, t, lo, hi):
    """4*sum w_i*eps_i for tile t with (unclamped) global window [lo, hi)."""
    lo_c, hi_c = max(0, lo), min(N, hi)
    rows = slice(t * 128, (t + 1) * 128)
    if hi_c <= lo_c:
        cov = np.zeros(128)
    else:
        cov = csum[rows, hi_c - 1] - (csum[rows, lo_c - 1] if lo_c > 0 else 0.0)
    eps = np.maximum(1.0 - cov / tot[rows], 0.0)
    return float((w_sorted[rows] * 4.0 * eps).sum())


def _core_shift(csum, tot, w_sorted, w, c, shard=SHARD):
    """Best per-core window shift (windows move together) + its bound."""
    best_s, best_b = 0, np.inf
    for s in range(-128, 129, 8):
        bc = 0.0
        for rt in range(shard // 128):
            t = (c * shard) // 128 + rt
            lo = t * 128 + 64 - w // 2 - s
            bc += _tile_bound(csum, tot, w_sorted, t, lo, lo + w)
        if bc < best_b:
            best_s, best_b = s, bc
    return best_s, best_b


def _band_err_bound(pts_sorted, w_sorted, w):
    """Zero-shift bound (fallback path check)."""
    csum, tot = _band_geometry(pts_sorted)
    return sum(_tile_bound(csum, tot, w_sorted, t,
                           t * 128 + 64 - w // 2, t * 128 + 64 + w // 2)
               for t in range(N // 128))


def _prep_inputs(points, weights, pointfea1, pointfea2,
                 nb=B, n=N, shard=SHARD, ncores=NCORES):
    """Host-side sort/shard/layout prep -> (w, list of per-core input dicts)."""
    points = np.asarray(points, np.float64)
    weights = np.asarray(weights, np.float64)
    f1 = np.asarray(pointfea1, np.float64)
    f2 = np.asarray(pointfea2, np.float64)

    rts = shard // 128
    per_batch = []
    geos = []
    for b in range(nb):
        order = _rcm_order(points[b])
        p = points[b][order]
        per_batch.append((order, p, weights[b][order], f1[b][order],
                          f2[b][order]))
        geos.append(_band_geometry(p))

    # smallest window whose per-core-optimally-shifted coverage bound passes
    w_band, shifts = 1024, np.zeros((nb, ncores), np.int64)
    # per-candidate acceptance threshold on the summed dropped-mass bound.
    # The bound is empirically >=20x conservative vs true loss error (row
    # errors cancel; max|P-Q| << 1 for dropped pairs), so even the 1.5e-2
    # tier keeps >20x true margin against the 2e-2 tolerance.
    for w_try, thresh in ((288, 1.5e-2), (320, 4e-3), (352, 4e-3),
                          (448, 4e-3)):
        cand = np.zeros((nb, ncores), np.int64)
        bound = 0.0
        for b in range(nb):
            csum, tot = geos[b]
            for c in range(ncores):
                s, bc = _core_shift(csum, tot, per_batch[b][2], w_try, c)
                cand[b, c] = s
                bound += bc
        if bound < thresh:
            w_band, shifts = w_try, cand
            break
    if w_band > 512:
        for (order, p, w_, _, _) in per_batch:
            assert _band_err_bound(p, w_, w_band) < 5e-3, \
                "band window too narrow for data"

    rp_all = np.empty((nb, KP, n), bf16)
    rf_all = np.empty((nb, KF, n), bf16)
    lp_all = np.empty((nb, KP, n), bf16)   # full rows; sliced per core
    lf_all = np.empty((nb, KF, n), bf16)
    wv_all = np.empty((nb, n), np.float32)

    for b, (order, p, w_, a, c) in enumerate(per_batch):
        xs = p / SIGMA
        x2 = (xs * xs).sum(-1)
        xh, xm, xl = _split(xs, 3)
        y2h, y2m, y2l = _split(x2, 3)
        bph, bpm, bpl = _split(-x2, 3)

        for d in range(3):
            rrows = [2 * xh[..., d], 2 * xm[..., d], 2 * xh[..., d],
                     2 * xl[..., d], 2 * xh[..., d], 2 * xm[..., d]]
            lrows = [xh[..., d], xh[..., d], xm[..., d],
                     xh[..., d], xl[..., d], xm[..., d]]
            for k in range(6):
                rp_all[b, 6 * d + k] = rrows[k]
                lp_all[b, 6 * d + k] = lrows[k]
        rp_all[b, 18] = -y2h
        rp_all[b, 19] = -y2m
        rp_all[b, 20] = -y2l
        rp_all[b, 21:24] = np.float32(1.0)
        lp_all[b, 18:21] = np.float32(1.0)
        lp_all[b, 21] = bph
        lp_all[b, 22] = bpm
        lp_all[b, 23] = bpl

        g1 = (a * a).sum(-1)
        g2 = (c * c).sum(-1)
        f1h, f1l = _split(a, 2)
        f2h, _ = _split(c, 2)
        g2h, g2m = _split(g2, 2)
        bfh, bfm = _split(np.float64(SHIFT_F) - g1, 2)

        rf_all[b, :64] = (2 * f2h.astype(np.float32)).T
        rf_all[b, 64:64 + NLO] = (2 * f2h[:, :NLO].astype(np.float32)).T
        rf_all[b, 124] = -g2h
        rf_all[b, 125] = -g2m
        rf_all[b, 126:128] = np.float32(1.0)
        lf_all[b, :64] = f1h.T
        lf_all[b, 64:64 + NLO] = f1l[:, :NLO].T
        lf_all[b, 124:126] = np.float32(1.0)
        lf_all[b, 126] = bfh
        lf_all[b, 127] = bfm

        wv_all[b] = w_.astype(np.float32)

    in_maps = []
    for cidx in range(ncores):
        r0 = cidx * shard
        sl = slice(r0, r0 + shard)
        rp = np.empty_like(rp_all)
        rf = np.empty_like(rf_all)
        for b in range(nb):
            # per-(batch,core) roll: aligns this core's rows to the fixed
            # rotated-space windows, plus the data-optimized shift
            roll = ROT_BASE - r0 + int(shifts[b, cidx])
            rp[b] = np.roll(rp_all[b], roll, axis=1)
            rf[b] = np.roll(rf_all[b], roll, axis=1)
        rp = np.ascontiguousarray(rp)
        rf = np.ascontiguousarray(rf)
        lpf = np.zeros((nb, 128, 2 * shard), bf16)
        lpf[:, :, :shard] = lf_all[:, :, sl]
        lpf[:, :KP, shard:] = lp_all[:, :, sl]
        wvt = np.empty((128, nb * rts), np.float32)
        for b in range(nb):
            for rt in range(rts):
                wvt[:, b * rts + rt] = \
                    wv_all[b, r0 + rt * 128:r0 + (rt + 1) * 128]
        in_maps.append({"rp": rp, "rf": rf,
                        "lpf": np.ascontiguousarray(lpf), "wv": wvt})
    return w_band, in_maps


def kernel(points, weights, pointfea1, pointfea2):
    global _last_results
    from concourse.bass_utils import run_bass_kernel_spmd

    w_band, in_maps = _prep_inputs(points, weights, pointfea1, pointfea2)
    key = f"nc_w{w_band}"
    if key not in _cache:
        _cache[key] = _build_program(w_band)
        _cache["nc"] = _cache[key]  # for test harness TimelineSim
    nc = _cache[key]
    _cache["nc"] = nc

    res = run_bass_kernel_spmd(nc, in_maps, core_ids=list(range(NCORES)))
    _last_results = res
    rts = RT_PER_BATCH
    out = np.zeros(B, np.float64)
    for c in range(NCORES):
        li = np.asarray(res.results[c]["out"], np.float64)  # [128, B*rts]
        for b in range(B):
            out[b] += li[:, b * rts:(b + 1) * rts].sum()
    return out.astype(np.float32)


if __name__ == "__main__":
    rng = np.random.default_rng(0)
    pts = rng.random((B, N, 3), np.float32)
    w = rng.random((B, N), np.float32)
    w /= w.sum(1, keepdims=True)
    a = rng.standard_normal((B, N, D)).astype(np.float32)
    bfea = rng.standard_normal((B, N, D)).astype(np.float32)
    out = kernel(pts, w, a, bfea)
    print("kernel out:", out)
